# revision 29
# baseline (speedup 1.0000x reference)
"""Trainium2 Bass kernel for nn_Decoder (GRU decoder + vocab projection + log_softmax).

Strategy (8 NeuronCores, SPMD single program):
  - Teacher forcing means all input tokens are known upfront: the embedding
    gather + relu happens on host; X^T ships to the device as bf16.
  - Phase 1 (replicated on every core): gi = X @ w_ih^T batched over all
    30*64 rows, then the sequential GRU recurrence over 30 steps with
    bf16 matmuls (fp32 PSUM accumulate) and fp32 gate math.  Each step's
    h_t is transposed on the PE into a [hidden, batch] bf16 slab.
  - Phase 2 (sharded by batch): each core computes full-vocab logits +
    log_softmax for 8 of the 64 batch rows.  SPMD cores run one program,
    so the shard is selected by *data*: core c receives X^T/h0 with the
    batch axis rotated by 8c, and statically uses local batches 0..7.
    No collectives, no dynamic addressing.
  - log_softmax without max-shift (logits are bounded ~|2|, exp is safe in
    fp32): logprob = l - ln(sum(exp(l))), with exp+row-sum fused in one
    ScalarE activation pass (accum_out).
"""

import sys

sys.path.insert(0, "/opt/trn_rl_repo")

import numpy as np
import ml_dtypes

import concourse.bass as bass
import concourse.mybir as mybir
import concourse.tile as tile
from concourse.bass_utils import run_bass_kernel_spmd
from concourse.masks import make_identity

F32 = mybir.dt.float32
FP8 = mybir.dt.float8e4
BF16 = mybir.dt.bfloat16
AL = mybir.AluOpType
ACT = mybir.ActivationFunctionType

P = 128


class Cfg:
    def __init__(self, hid=1024, voc=32000, steps=30, batch=64, ncores=8,
                 nchunk=4, vchunk=500, vblock=4, gi_bufs=2, ngr=32,
                 debug=False, wout_fp8=False):
        self.wout_fp8 = wout_fp8
        self.debug = debug
        self.hid = hid
        self.voc = voc
        self.steps = steps
        self.batch = batch
        self.ncores = ncores
        self.bl = batch // ncores          # local batch rows per core
        self.kt = hid // P                 # hidden k-tiles
        self.g3 = 3 * hid
        self.nchunk = nchunk               # gate chunks per step
        self.ch = hid // nchunk            # hidden dims per chunk
        self.gcl = 3 * self.ch             # gate-chunk col width (regrouped)
        self.rows = steps * batch
        self.mtiles = self.rows // P
        self.lrows = steps * self.bl          # local rows (this core's shard)
        self.lmtiles = (self.lrows + P - 1) // P       # gi m-tiles
        self.vchunk = vchunk               # logits n-chunk
        self.nvc = voc // vchunk
        self.vblock = vblock               # chunks per lhsT-reuse block
        self.gi_bufs = gi_bufs
        self.ngr = ngr                     # exp groups
        # phase-2 m-tiles over (t, local batch)
        self.tpm = P // self.bl            # steps per full m-tile
        self.p2m = []
        t0 = 0
        while t0 < steps:
            tl = min(self.tpm, steps - t0)
            self.p2m.append((t0, tl))
            t0 += tl


def gate_col_perm(cfg):
    """Column regroup for w_ih^T / w_hh^T / gate bias: chunk-major
    [r_c | z_c | n_c] blocks so each gate chunk is contiguous."""
    idx = []
    for c in range(cfg.nchunk):
        for g in range(3):
            s = g * cfg.hid + c * cfg.ch
            idx.extend(range(s, s + cfg.ch))
    return np.asarray(idx)


def _split_multi_waits(nc):
    """This toolchain's walrus encodes at most ONE sync wait per instruction.
    Tile emits several; hoist extras onto single-wait NoOps just before."""
    n_split = 0
    for f in nc.m.functions:
        for blk in f.blocks:
            insts = blk.instructions
            idx = 0
            while idx < len(insts):
                inst = insts[idx]
                si = inst.sync_info
                if si is not None and si.on_wait and len(si.on_wait) > 1:
                    waits = list(si.on_wait)
                    for w in waits[:-1]:
                        nop = mybir.InstNoOp(
                            name=nc.get_next_instruction_name(),
                            engine=inst.engine,
                            bass_nofuse=True,
                            sync_info=mybir.SyncInfo(on_wait=[w], on_update=[]),
                        )
                        nc.register_instruction(nop, overwrite=True)
                        insts.insert(idx, nop)
                        idx += 1
                    inst.sync_info = mybir.SyncInfo(
                        on_wait=[waits[-1]], on_update=list(si.on_update))
                    n_split += 1
                idx += 1
    return n_split


def build_kernel(cfg: Cfg):
    nc = bass.Bass()
    B = cfg.batch

    LB = cfg.bl
    xt = nc.declare_dram_parameter("xt", [cfg.lmtiles, P, cfg.kt, P], BF16,
                                   isOutput=False)
    h0t = nc.declare_dram_parameter("h0t", [P, cfg.kt, LB], BF16, isOutput=False)
    h0f = nc.declare_dram_parameter("h0f", [LB, cfg.hid], F32, isOutput=False)
    wih = nc.declare_dram_parameter("wih", [P, cfg.kt, cfg.g3], BF16,
                                    isOutput=False)
    whh = nc.declare_dram_parameter("whh", [P, cfg.kt, cfg.g3], BF16,
                                    isOutput=False)
    WDT = FP8 if cfg.wout_fp8 else BF16
    wout = nc.declare_dram_parameter(
        "wout", [cfg.nvc, P, cfg.kt, cfg.vchunk], WDT, isOutput=False)
    gbias = nc.declare_dram_parameter("gbias", [P, cfg.g3], BF16, isOutput=False)
    bhhn = nc.declare_dram_parameter("bhhn", [1, cfg.hid], BF16,
                                     isOutput=False)
    nblk0 = cfg.nvc // cfg.vblock
    ob = nc.declare_dram_parameter(
        "ob", [nblk0, P, cfg.vblock, cfg.vchunk], BF16, isOutput=False)

    lp = nc.declare_dram_parameter(
        "lp", [cfg.steps * cfg.bl, cfg.voc], F32, isOutput=True)
    hfin = nc.declare_dram_parameter("hfin", [LB, cfg.hid], F32, isOutput=True)
    hdbg = None
    if getattr(cfg, "debug", False):
        hdbg = nc.declare_dram_parameter(
            "hdbg", [P, cfg.kt, cfg.steps + 1, LB], BF16, isOutput=True)


    with tile.TileContext(nc) as tc:
        with (
            tc.tile_pool(name="const", bufs=1) as const_pool,
            tc.tile_pool(name="mcomp", bufs=1) as mcomp_pool,
        ):
            ident = const_pool.tile([LB, LB], F32, tag="ident")
            make_identity(nc, ident[:])
            ident8b = const_pool.tile([LB, LB], BF16, tag="ident8b")
            make_identity(nc, ident8b[:])
            ones8 = const_pool.tile([1, LB], BF16, tag="ones8")
            nc.gpsimd.memset(ones8[:], 1.0)

            mcs = []  # phase-2 lhsT tiles (filled after recurrence)

            with tc.tile_pool(name="hT", bufs=1) as hT_pool:
                # h^T slab: slot s holds h_{s-1}; slot 0 = h0
                hT = hT_pool.tile([P, cfg.kt, cfg.steps + 1, LB], BF16, tag="hT")
                nc.gpsimd.dma_start(hT[:, :, 0, :], h0t[:])

                # ------------- phase 1: gi + recurrence -------------
                with (
                    tc.tile_pool(name="wslab", bufs=1) as wslab_pool,
                    tc.tile_pool(name="gi", bufs=cfg.lmtiles) as gi_pool,
                    tc.tile_pool(name="gst", bufs=3) as gst_pool,
                    tc.tile_pool(name="gps", bufs=1, space="PSUM") as gips_pool,
                    tc.tile_pool(name="ghps", bufs=2, space="PSUM") as ghps_pool,
                    tc.tile_pool(name="tpps", bufs=1, space="PSUM") as tpps_pool,
                    tc.tile_pool(name="xst", bufs=2) as x_pool,
                    tc.tile_pool(name="gtmp", bufs=2) as gtmp_pool,
                    tc.tile_pool(name="hcur", bufs=2) as hcur_pool,
                ):
                    gb_sb = wslab_pool.tile([P, cfg.g3], BF16, tag="gb")
                    nc.gpsimd.dma_start(gb_sb[:], gbias[:])
                    bhh_sb = wslab_pool.tile([1, cfg.hid], BF16, tag="bhhn")
                    nc.gpsimd.dma_start(bhh_sb[:], bhhn[:])
                    prb = wslab_pool.tile([1, 4], F32, tag="prb")

                    def dve_probe(ap):
                        # absorb a DMA-completion wait into one cheap DVE op so
                        # later DVE instructions inherit it via engine order
                        nc.vector.tensor_copy(prb[0:1, 0:1], ap)

                    x_tiles = []
                    for mt in range(cfg.lmtiles):
                        x_sb = x_pool.tile([P, cfg.kt, P], BF16, tag="x",
                                           name=f"x_{mt}")
                        nc.sync.dma_start(x_sb[:], xt[mt])
                        x_tiles.append(x_sb)
                    wih_sb = wslab_pool.tile([P, cfg.kt, cfg.g3], BF16, tag="wih")
                    whh_sb = wslab_pool.tile([P, cfg.kt, cfg.g3], BF16, tag="whh")
                    for kt in range(cfg.kt):
                        nc.sync.dma_start(wih_sb[:, kt], wih[:, kt])
                        nc.scalar.dma_start(whh_sb[:, kt], whh[:, kt])

                    h_cur = hcur_pool.tile([LB, cfg.hid], F32, tag="hc")
                    nc.gpsimd.dma_start(h_cur[:], h0f[:])
                    dve_probe(gb_sb[0:1, 0:1])
                    dve_probe(bhh_sb[0:1, 0:1])

                    def emit_gi_mtile(mt):
                        x_sb = x_tiles[mt]
                        gi_sb = gi_pool.tile([P, cfg.g3], F32, tag="gi",
                                             name=f"gi_{mt}")
                        nsub = cfg.g3 // 512
                        for sub in range(nsub):
                            c0 = sub * 512
                            ps = gips_pool.tile([P, 512], F32, tag="gips",
                                                name=f"gips_{mt}_{sub}")
                            for kt in range(cfg.kt):
                                nc.tensor.matmul(
                                    ps[:], x_sb[:, kt, :],
                                    wih_sb[:, kt, c0:c0 + 512],
                                    start=(kt == 0), stop=(kt == cfg.kt - 1))
                            nc.vector.tensor_tensor(
                                gi_sb[:, c0:c0 + 512], ps[:],
                                gb_sb[:, c0:c0 + 512],
                                AL.add)
                        return gi_sb

                    spm = P // LB                  # steps per gi m-tile
                    gi_tiles = [emit_gi_mtile(mt) for mt in range(cfg.lmtiles)]

                    for t in range(cfg.steps):
                        mt = t // spm
                        po = (t % spm) * LB
                        # this step's gi rows sit at partitions po..po+LB;
                        # DVE lanes cannot cross partitions -> DMA-bounce to 0
                        giv_t = gst_pool.tile([LB, cfg.g3], BF16, tag="gst",
                                              name=f"gst_{t}")
                        nc.gpsimd.dma_start(giv_t[:], gi_tiles[mt][po:po + LB, :])
                        dve_probe(giv_t[0:1, 0:1])
                        h_nxt = hcur_pool.tile([LB, cfg.hid], F32, tag="hc",
                                               name=f"h_{t}")
                        for c in range(cfg.nchunk):
                            gl = c * cfg.gcl
                            ps = ghps_pool.tile([LB, cfg.gcl], F32, tag="ghps",
                                                name=f"ghps_{t}_{c}")
                            ch = cfg.ch
                            giv = giv_t[0:LB, gl:gl + cfg.gcl]
                            # region splits aligned between the kt-matmuls and
                            # the accumulating gi/bias matmuls below
                            gsubs = [(s, min(512, 2 * ch - s))
                                     for s in range(0, 2 * ch, 512)]
                            gsubs += [(2 * ch + s, min(512, ch - s))
                                      for s in range(0, ch, 512)]
                            for kt in range(cfg.kt):
                                for s0, sw in gsubs:
                                    nc.tensor.matmul(
                                        ps[:, s0:s0 + sw], hT[:, kt, t, :],
                                        whh_sb[:, kt, gl + s0:gl + s0 + sw],
                                        start=(kt == 0), stop=False)
                            # accumulate gi (r,z cols) via identity matmul, and
                            # b_hh (n cols, pre-r-gating) via a ones-row matmul
                            for s0, sw in gsubs:
                                if s0 < 2 * ch:
                                    nc.tensor.matmul(
                                        ps[:, s0:s0 + sw], ident8b[:],
                                        giv[:, s0:s0 + sw],
                                        start=False, stop=True)
                                else:
                                    nc.tensor.matmul(
                                        ps[:, s0:s0 + sw], ones8[:],
                                        bhh_sb[:, c * ch + s0 - 2 * ch:
                                               c * ch + s0 - 2 * ch + sw],
                                        start=False, stop=True)
                            r = gtmp_pool.tile([LB, ch], F32, tag="r")
                            z = gtmp_pool.tile([LB, ch], F32, tag="z")
                            nc.scalar.activation(r[:], ps[:, 0:ch], ACT.Sigmoid)
                            nc.scalar.activation(z[:], ps[:, ch:2 * ch], ACT.Sigmoid)
                            t1 = gtmp_pool.tile([LB, ch], F32, tag="t1")
                            nc.vector.tensor_mul(t1[:], r[:], ps[:, 2 * ch:3 * ch])
                            nc.vector.tensor_add(t1[:], t1[:], giv[:, 2 * ch:3 * ch])
                            n = gtmp_pool.tile([LB, ch], F32, tag="n")
                            nc.scalar.activation(n[:], t1[:], ACT.Tanh)
                            cs = c * ch
                            d = gtmp_pool.tile([LB, ch], F32, tag="d")
                            nc.vector.tensor_sub(d[:], h_cur[:, cs:cs + ch], n[:])
                            nc.vector.tensor_mul(d[:], z[:], d[:])
                            nc.vector.tensor_add(h_nxt[:, cs:cs + ch], n[:], d[:])
                            nk = max(1, ch // P)
                            tp = tpps_pool.tile([P, nk, LB], F32, tag="tp")
                            for u in range(nk):
                                kti = (cs + u * P) // P
                                nc.tensor.transpose(
                                    tp[:, u, :],
                                    h_nxt[:, kti * P:kti * P + P], ident[:])
                            nc.vector.tensor_copy(
                                hT[:, (cs // P):(cs // P) + nk, t + 1, :], tp[:])
                        h_cur = h_nxt

                    nc.sync.dma_start(hfin[:], h_cur[:])

                if hdbg is not None:
                    nc.sync.dma_start(hdbg[:], hT[:])

                # ------- compact phase-2 lhsT tiles out of the slab -------
                for mi, (t0, tl) in enumerate(cfg.p2m):
                    mc = mcomp_pool.tile([P, cfg.kt, P], WDT, tag=f"mc{mi}")
                    nc.vector.tensor_copy(
                        mc.rearrange("p k (t b) -> p k t b", b=cfg.bl)[:, :, 0:tl, :],
                        hT[:, :, 1 + t0:1 + t0 + tl, 0:cfg.bl])
                    mcs.append((mc, tl * cfg.bl))

            # ------------- phase 2: logits + log_softmax -------------
            with (
                tc.tile_pool(name="wv", bufs=cfg.vblock + 2) as wv_pool,
                tc.tile_pool(name="obp", bufs=1) as obp_pool,
                tc.tile_pool(name="lps", bufs=2 * cfg.vblock, space="PSUM") as lps_pool,
                tc.tile_pool(name="lslab", bufs=len(cfg.p2m)) as lslab_pool,
                tc.tile_pool(name="junk", bufs=1) as junk_pool,
                tc.tile_pool(name="stats", bufs=4) as stats_pool,
                tc.tile_pool(name="stg", bufs=2) as stg_pool,
            ):
                slabs = [lslab_pool.tile([P, cfg.voc], BF16, tag="lslab",
                                         name=f"lslab{i}")
                         for i in range(len(cfg.p2m))]

                nblk = cfg.nvc // cfg.vblock
                bw = cfg.vblock * cfg.vchunk
                accs = [stats_pool.tile([P, nblk], F32, tag="acc",
                                        name=f"acc{i}")
                        for i in range(len(cfg.p2m))]
                for blk in range(nblk):
                    obt = obp_pool.tile([P, cfg.vblock, cfg.vchunk], BF16, tag="obt")
                    nc.gpsimd.dma_start(obt[:], ob[blk])
                    wts = []
                    for ci in range(cfg.vblock):
                        wv = wv_pool.tile([P, cfg.kt, cfg.vchunk], WDT, tag="wv")
                        eng = nc.sync if ci % 2 == 0 else nc.gpsimd
                        eng.dma_start(wv[:], wout[blk * cfg.vblock + ci])
                        wts.append(wv)
                    for mi, (mc, mrows) in enumerate(mcs):
                        pss = [lps_pool.tile([P, cfg.vchunk], F32, tag="lps",
                                             name=f"lps_{blk}_{mi}_{ci}")
                               for ci in range(cfg.vblock)]
                        for kt in range(cfg.kt):
                            for ci in range(cfg.vblock):
                                nc.tensor.matmul(
                                    pss[ci][0:mrows, :], mc[:, kt, 0:mrows],
                                    wts[ci][:, kt, :],
                                    start=(kt == 0), stop=(kt == cfg.kt - 1))
                        for ci in range(cfg.vblock):
                            cidx = blk * cfg.vblock + ci
                            n0 = cidx * cfg.vchunk
                            nc.vector.tensor_tensor(
                                slabs[mi][0:mrows, n0:n0 + cfg.vchunk],
                                pss[ci][0:mrows, :],
                                obt[0:mrows, ci, :],
                                AL.add)
                        junk = junk_pool.tile([P, bw], BF16, tag="junk",
                                              name=f"junk_{blk}_{mi}")
                        nc.scalar.activation(
                            junk[0:mrows, :],
                            slabs[mi][0:mrows, blk * bw:(blk + 1) * bw],
                            ACT.Exp, accum_out=accs[mi][0:mrows, blk:blk + 1])

                for mi, (mc, mrows) in enumerate(mcs):
                    lslab = slabs[mi]
                    ssum = stats_pool.tile([P, 1], F32, tag="ssum",
                                           name=f"ssum{mi}")
                    nc.vector.reduce_sum(ssum[0:mrows, :], accs[mi][0:mrows, :],
                                         axis=mybir.AxisListType.X)
                    nlz = stats_pool.tile([P, 1], F32, tag="nlz", name=f"nlz{mi}")
                    nc.scalar.activation(nlz[0:mrows, :], ssum[0:mrows, :], ACT.Ln)
                    nc.vector.tensor_scalar_mul(nlz[0:mrows, :], nlz[0:mrows, :], -1.0)
                    for g in range(nblk):
                        n0 = g * bw
                        stg = stg_pool.tile([P, bw], F32, tag="stg",
                                            name=f"stg_{mi}_{g}")
                        nc.vector.tensor_scalar(
                            stg[0:mrows, :], lslab[0:mrows, n0:n0 + bw],
                            nlz[0:mrows, :], None, AL.add)
                        deng = (nc.sync, nc.scalar, nc.gpsimd)[g % 3]
                        deng.dma_start(
                            lp[mi * P:mi * P + mrows, n0:n0 + bw],
                            stg[0:mrows, :])

    _split_multi_waits(nc)
    return nc


_CACHED = {}


def get_nc(cfg=None, key="full"):
    if key not in _CACHED:
        _CACHED[key] = (build_kernel(cfg or Cfg()), cfg or Cfg())
    return _CACHED[key]


def host_prep(cfg, target_tensor, embedding, encoder_hidden,
              w_ih, w_hh, b_ih, b_hh, out_w, out_b, X_pre=None):
    B, T, H = cfg.batch, cfg.steps, cfg.hid
    h0 = np.asarray(encoder_hidden, np.float32)[0]          # [B, H]

    if X_pre is not None:
        X = np.asarray(X_pre, np.float32)
    else:
        target = np.asarray(target_tensor)
        emb = np.asarray(embedding, np.float32)
        toks = np.concatenate(
            [np.zeros((B, 1), target.dtype), target[:, :-1]], axis=1)  # [B, T]
        X = np.maximum(emb[toks.T.astype(np.int64)], 0.0)   # [T, B, H] fp32

    perm_idx = gate_col_perm(cfg)
    kt = cfg.kt
    wih_t = np.ascontiguousarray(
        np.asarray(w_ih, np.float32).T[:, perm_idx]
        .reshape(kt, 128, cfg.g3).transpose(1, 0, 2)).astype(ml_dtypes.bfloat16)
    whh_t = np.ascontiguousarray(
        np.asarray(w_hh, np.float32).T[:, perm_idx]
        .reshape(kt, 128, cfg.g3).transpose(1, 0, 2)).astype(ml_dtypes.bfloat16)
    wdt = ml_dtypes.float8_e4m3 if getattr(cfg, "wout_fp8", False) \
        else ml_dtypes.bfloat16
    wout_t = np.ascontiguousarray(
        np.asarray(out_w, np.float32).T
        .reshape(kt, 128, cfg.nvc, cfg.vchunk)
        .transpose(2, 1, 0, 3)).astype(wdt)
    b_ih_f = np.asarray(b_ih, np.float32)
    b_hh_f = np.asarray(b_hh, np.float32)
    gb_full = b_ih_f + b_hh_f
    gb_full[2 * cfg.hid:] = b_ih_f[2 * cfg.hid:]   # n-gate: b_hh enters via r*(gh+b_hh)
    gb = gb_full[perm_idx]
    gb = np.ascontiguousarray(np.broadcast_to(gb[None, :], (128, cfg.g3))
                              ).astype(ml_dtypes.bfloat16)
    nblk0 = cfg.nvc // cfg.vblock
    obc = np.ascontiguousarray(
        np.broadcast_to(
            np.asarray(out_b, np.float32)
            .reshape(nblk0, 1, cfg.vblock, cfg.vchunk),
            (nblk0, 128, cfg.vblock, cfg.vchunk))).astype(ml_dtypes.bfloat16)

    bhhn_rep = np.ascontiguousarray(
        b_hh_f[2 * cfg.hid:][None, :]).astype(ml_dtypes.bfloat16)
    shared = dict(wih=wih_t, whh=whh_t, wout=wout_t, gbias=gb, ob=obc,
                  bhhn=bhhn_rep)
    in_maps = []
    for c in range(cfg.ncores):
        bl = cfg.bl
        Xl = X[:, c * bl:(c + 1) * bl, :].reshape(T * bl, H)   # rows t-major
        Xpad = np.zeros((cfg.lmtiles * 128, H), np.float32)
        Xpad[:T * bl] = Xl
        xtc = np.ascontiguousarray(
            Xpad.T.reshape(kt, 128, cfg.lmtiles, 128)
            .transpose(2, 1, 0, 3)).astype(ml_dtypes.bfloat16)
        h0p = np.ascontiguousarray(h0[c * bl:(c + 1) * bl])
        h0tc = np.ascontiguousarray(
            h0p.T.reshape(kt, 128, bl)
            .transpose(1, 0, 2)).astype(ml_dtypes.bfloat16)
        in_maps.append(dict(shared, xt=xtc, h0t=h0tc,
                            h0f=np.ascontiguousarray(h0p, np.float32)))
    return in_maps


def assemble(cfg, results):
    B, T, V = cfg.batch, cfg.steps, cfg.voc
    log_probs = np.empty((B, T, V), np.float32)
    for c in range(cfg.ncores):
        lpc = np.asarray(results[c]["lp"]).reshape(T, cfg.bl, V)
        log_probs[c * cfg.bl:(c + 1) * cfg.bl] = np.transpose(lpc, (1, 0, 2))
    hidden = np.concatenate(
        [np.asarray(results[c]["hfin"]) for c in range(cfg.ncores)])[None]
    return log_probs, hidden


def kernel(encoder_outputs, encoder_hidden, target_tensor, embedding,
           w_ih, w_hh, b_ih, b_hh, out_w, out_b):
    nc, cfg = get_nc()
    in_maps = host_prep(cfg, target_tensor, embedding, encoder_hidden,
                        w_ih, w_hh, b_ih, b_hh, out_w, out_b)
    res = run_bass_kernel_spmd(nc, in_maps, list(range(cfg.ncores)))
    return assemble(cfg, res.results)
